# revision 26
# baseline (speedup 1.0000x reference)
"""Trainium2 Bass kernel for causal multi-head attention with RoPE.

Problem: x[2,2048,2048] -> qkv proj -> RoPE(q,k) -> causal softmax attention
(16 heads, hd=128) -> out proj.  Sharding: tensor-parallel over heads
(2 heads/core x 8 cores); the output projection contraction is restored
with one AllToAll per (batch, head) (head-shards -> sequence-shards), so
each core computes a disjoint [2, 256, 2048] slice of the final output.

v2: all matmul operands are bf16 (PSUM accumulation stays fp32), which
makes LDWEIGHTS (107ns) hide completely under N=512 matmuls and halves
the AllToAll payload.  The causal mask is applied post-exp with a DVE
affine_select on the diagonal 128-block (no PE mask matmuls).  Softmax
denominators: PE ones-matmul partition-reduce -> DVE reciprocal ->
gpsimd partition_broadcast (no PE broadcast matmul).  RoPE is applied
per 512-chunk right after each projection copy so attention starts
almost immediately after the last chunk.  Batch-0's output projection
runs as filler groups inside batch-1's second attention head; all of
w_out is prefetched during attention so the tail is only the last
AllToAll + batch-1's projection.
"""

import os
import sys

if "/opt/trn_rl_repo" not in sys.path:
    sys.path.insert(0, "/opt/trn_rl_repo")

import numpy as np
import ml_dtypes

BF16 = ml_dtypes.bfloat16

B, S, D = 2, 2048, 2048
H, HD = 16, 128
NCORES = 8
HPC = H // NCORES          # heads per core (2)
ROPE_BASE = 10000.0
SCALE = 1.0 / float(np.sqrt(HD))
SC = 512                   # QKV matmul free-dim chunk (s positions)
KSUB = D // 128            # 16 contraction subtiles
SCW = S // NCORES          # 256: per-core output cols per batch
NQC = S // SC              # 4 qkv s-chunks per batch
NKT = S // 128             # 16 key tiles
VOFF = 2 * HPC * HD        # v block column offset in w shard (512)
FILL_KTS = (4, 5, 7, 8, 9, 11, 12, 13, 15)   # filler slots (avoid finalize kts)

_CACHE = {}


def _install_trace_shim():
    """Optionally register the axon NTFF profile hook (for test.py tracing)."""
    try:
        import types

        if "antenv.axon_hooks" in sys.modules:
            return True
        import antenv
        from trn_agent_boot.trn_boot import _ntff_profile_via_ctypes

        hook = _ntff_profile_via_ctypes("/opt/axon/libaxon_pjrt.so")
        mod = types.ModuleType("antenv.axon_hooks")
        _state = {"hook": hook}
        mod.get_axon_ntff_profile_hook = lambda: _state["hook"]
        mod.set_axon_ntff_profile_hook = lambda h: _state.__setitem__("hook", h)
        sys.modules["antenv.axon_hooks"] = mod
        antenv.axon_hooks = mod
        return True
    except Exception:
        return False


def _build():
    import concourse.bass as bass  # noqa: F401
    import concourse.mybir as mybir
    import concourse.tile as tile
    from concourse import bacc

    f32 = mybir.dt.float32
    f32r = mybir.dt.float32r
    bf16 = mybir.dt.bfloat16
    EXP = mybir.ActivationFunctionType.Exp

    nc = bacc.Bacc("TRN2", target_bir_lowering=False, debug=False,
                   num_devices=NCORES)

    xT = nc.dram_tensor("xT", [128, KSUB, B * S], bf16, kind="ExternalInput")
    wqkv = nc.dram_tensor("wqkv", [128, KSUB, 3 * HPC * HD], bf16,
                          kind="ExternalInput")
    wout = nc.dram_tensor("wout", [128, KSUB, D], bf16, kind="ExternalInput")
    cosg = nc.dram_tensor("cosg", [128, S], bf16, kind="ExternalInput")
    sing = nc.dram_tensor("sing", [128, S], bf16, kind="ExternalInput")
    mneg = nc.dram_tensor("mneg", [128, 512], bf16, kind="ExternalInput")
    y = nc.dram_tensor("y", [B, SCW, D], f32, kind="ExternalOutput")

    with tile.TileContext(nc) as tc:
        with tc.tile_pool(name="const", bufs=1) as cp, \
             tc.tile_pool(name="dram", bufs=1, space="DRAM") as dp, \
             tc.tile_pool(name="psO", bufs=1, space="PSUM") as psO, \
             tc.tile_pool(name="psA", bufs=2, space="PSUM") as psA, \
             tc.tile_pool(name="psD", bufs=1, space="PSUM") as psD, \
             tc.tile_pool(name="psF", bufs=1, space="PSUM") as psF, \
             tc.tile_pool(name="w", bufs=1) as wp, \
             tc.tile_pool(name="xc", bufs=2) as xp, \
             tc.tile_pool(name="qkv", bufs=1) as qp, \
             tc.tile_pool(name="attn", bufs=1) as ap_, \
             tc.tile_pool(name="stp", bufs=1) as stp, \
             tc.tile_pool(name="rot", bufs=1) as rp, \
             tc.tile_pool(name="exp", bufs=6) as ep, \
             tc.tile_pool(name="row", bufs=2) as sp_, \
             tc.tile_pool(name="lhs", bufs=2) as lp, \
             tc.tile_pool(name="wo", bufs=4) as wop, \
             tc.tile_pool(name="part", bufs=8) as pp, \
             tc.tile_pool(name="ys", bufs=2) as yp:

            from concourse.masks import make_identity

            cos_sb = cp.tile([128, S], bf16, name="cos_sb")
            sin_sb = cp.tile([128, S], bf16, name="sin_sb")
            mneg_sb = cp.tile([128, 512], bf16, name="mneg_sb")
            identF = cp.tile([128, 128], f32, name="identF")
            identB = cp.tile([128, 128], bf16, name="identB")
            onesc = cp.tile([128, 1], f32, name="onesc")
            onescR = cp.tile([128, 1], f32r, name="onescR")
            onesr = cp.tile([1, 128], bf16, name="onesr")
            nc.vector.memset(onesc[:], 1.0)
            nc.vector.tensor_copy(onescR[:], onesc[:])
            nc.vector.memset(onesr[:], 1.0)
            make_identity(nc, identF[:])
            nc.vector.tensor_copy(identB[:], identF[:])

            # batch-0 chunk-0 x and the whole qkv weight shard: both issued
            # from gpsimd (software DGE — descriptor generation is ~20x
            # faster than the sync queue's HWDGE for 2048-descriptor loads).
            xc0 = xp.tile([128, KSUB, SC], bf16, tag="xc", name="xc")
            nc.gpsimd.dma_start(xc0[:], xT.ap()[:, :, 0:SC])
            wAll = wp.tile([128, KSUB, 3 * HPC * HD], bf16, name="wAll")
            nc.gpsimd.dma_start(wAll[:], wqkv.ap())
            nc.scalar.dma_start(cos_sb[:], cosg.ap())
            nc.scalar.dma_start(sin_sb[:], sing.ap())
            nc.scalar.dma_start(mneg_sb[:], mneg.ap())

            ibs = {(b, h): dp.tile([NCORES, 128, SCW], bf16, name=f"ib{b}{h}")
                   for b in range(B) for h in range(HPC)}
            obs = {(b, h): dp.tile([NCORES, 128, SCW], bf16, name=f"ob{b}{h}")
                   for b in range(B) for h in range(HPC)}

            def qkv_rope(b, pre_xc):
                qkT = qp.tile([128, 2 * HPC, S], bf16, tag="qkT")
                Vn = qp.tile([128, NKT, HPC * HD], bf16, tag="Vn")
                xcs = [pre_xc]
                for sc in range(NQC):
                    xc = xcs[sc]
                    if sc + 1 < NQC:
                        # prefetch next chunk (gpsimd queue, one ahead)
                        nxt = xp.tile([128, KSUB, SC], bf16, tag="xc",
                                      name="xc")
                        off = b * S + (sc + 1) * SC
                        nc.gpsimd.dma_start(nxt[:],
                                            xT.ap()[:, :, off:off + SC])
                        xcs.append(nxt)
                    sl = slice(sc * SC, (sc + 1) * SC)
                    for m in (0, 2, 1, 3):   # q0, k0, q1, k1
                        ps = psA.tile([128, 512], f32, tag="bank")
                        for k in range(KSUB):
                            nc.tensor.matmul(
                                ps[:, :SC],
                                wAll[:, k, m * 128:(m + 1) * 128],
                                xc[:, k],
                                start=(k == 0), stop=(k == KSUB - 1))
                        nc.vector.tensor_copy(qkT[:, m, sl], ps[:, :SC])
                        # RoPE, fused halves (sin grid stored pre-swapped):
                        # rt[0:64] = t[64:128]*(-sin); rt[64:128] = t[0:64]*sin
                        # t *= cos; t += rt
                        rt = rp.tile([128, SC], bf16, tag="rot", name="rt")
                        nc.vector.tensor_mul(rt[0:64, :],
                                             qkT[64:128, m, sl],
                                             sin_sb[64:128, sl])
                        nc.vector.tensor_mul(rt[64:128, :],
                                             qkT[0:64, m, sl],
                                             sin_sb[0:64, sl])
                        nc.vector.tensor_mul(qkT[:, m, sl], qkT[:, m, sl],
                                             cos_sb[:, sl])
                        nc.vector.tensor_add(qkT[:, m, sl], qkT[:, m, sl],
                                             rt[:])
                    for st2 in range(SC // 128):
                        ps = psA.tile([128, 512], f32, tag="bank")
                        for k in range(KSUB):
                            nc.tensor.matmul(
                                ps[:, :HPC * HD],
                                xc[:, k, st2 * 128:(st2 + 1) * 128],
                                wAll[:, k, VOFF:VOFF + HPC * HD],
                                start=(k == 0), stop=(k == KSUB - 1))
                        nc.vector.tensor_copy(Vn[:, sc * (SC // 128) + st2],
                                              ps[:, :HPC * HD])
                return qkT, Vn

            def attention(b, h, qkT, Vn, fillers=()):
                fillers = list(fillers)
                outT = psO.tile([128, S], f32, tag="outT")
                acc = ap_.tile([128, S], f32r, tag="acc")
                st = stp.tile([128, S], bf16, tag="st")

                def emit_av(kt, off, ets):
                    q0 = 512 * (kt // 4)
                    for c in range(len(ets)):
                        qs = q0 + c * 512
                        o = off if c == 0 else 0
                        nc.tensor.matmul(
                            outT[:, qs + o:qs + 512],
                            Vn[:, kt, h * 128:(h + 1) * 128],
                            ets[c][:, o:512],
                            start=(kt == 0),
                            stop=(kt == 4 * (qs // 512) + 3))

                def finalize_chunk(j):
                    # denom: partition-reduce ones-matmul, fp32 reciprocal,
                    # K=1 broadcast matmul, normalize, ship to DRAM.
                    sl = slice(j * 512, (j + 1) * 512)
                    rps = psD.tile([128, 512], f32, tag="dps", name="rps")
                    nc.tensor.matmul(rps[0:1, :], onescR[:], acc[:, sl],
                                     start=True, stop=True)
                    srow = sp_.tile([1, 512], f32, tag="srow")
                    nc.vector.reciprocal_approx_fast(srow[:], rps[0:1, :])
                    srb = sp_.tile([1, 512], bf16, tag="srb")
                    nc.vector.tensor_copy(srb[:], srow[:])
                    bp = psD.tile([128, 512], f32, tag="dps", name="bp")
                    nc.tensor.matmul(bp[:], onesr[:], srb[:],
                                     start=True, stop=True)
                    nc.vector.tensor_copy(st[:, sl], outT[:, sl])
                    nc.vector.tensor_mul(st[:, sl], st[:, sl], bp[:])
                    for jj in (2 * j, 2 * j + 1):
                        nc.gpsimd.dma_start(ibs[(b, h)][jj],
                                            st[:, jj * SCW:(jj + 1) * SCW])

                def emit_score(kt, c, off):
                    q0 = 512 * (kt // 4)
                    qs = q0 + c * 512
                    o = off if c == 0 else 0
                    sp = psA.tile([128, 512], f32, tag="bank")
                    if c == 0:
                        # -1e9 upper-tri mask for the diagonal 128 block
                        nc.tensor.matmul(sp[:, o:512], identB[:],
                                         mneg_sb[:, 0:512 - o],
                                         start=True, stop=False)
                    nc.tensor.matmul(
                        sp[:, o:512],
                        qkT[:, HPC + h, kt * 128:(kt + 1) * 128],
                        qkT[:, h, qs + o:qs + 512],
                        start=(c != 0), stop=True)
                    return sp

                prev = None
                for kt in range(NKT):
                    q0 = 512 * (kt // 4)
                    off = 128 * (kt % 4)
                    nch = (S - q0) // 512
                    sps = [emit_score(kt, c, off)
                           for c in range(min(nch, 2))]
                    if prev is not None:
                        emit_av(*prev)
                    sps += [emit_score(kt, c, off)
                            for c in range(2, nch)]
                    if kt >= 6 and (kt - 6) % 4 == 0:
                        finalize_chunk((kt - 6) // 4)
                    ets = []
                    for c in range(nch):
                        o = off if c == 0 else 0
                        et = ep.tile([128, 512], bf16, tag="expT")
                        ets.append(et)
                        nc.scalar.activation(et[:, o:512], sps[c][:, o:512],
                                             EXP, scale=SCALE)
                    for c in range(nch):
                        qs = q0 + c * 512
                        o = off if c == 0 else 0
                        if kt == 0:
                            nc.vector.tensor_copy(acc[:, qs:qs + 512],
                                                  ets[c][:])
                        else:
                            eng = nc.gpsimd if qs // 512 < 2 else nc.vector
                            eng.tensor_add(acc[:, qs + o:qs + 512],
                                           acc[:, qs + o:qs + 512],
                                           ets[c][:, o:512])
                    if fillers and kt in FILL_KTS:
                        fillers.pop(0)()
                    prev = (kt, off, ets)
                emit_av(*prev)
                finalize_chunk(3)
                while fillers:
                    fillers.pop(0)()

            def a2a(b, h):
                nc.gpsimd.collective_compute(
                    "AllToAll", mybir.AluOpType.bypass,
                    replica_groups=[list(range(NCORES))],
                    ins=[ibs[(b, h)].opt()], outs=[obs[(b, h)].opt()])

            def load_lhs_part(b, hh, lhs):
                # k-subtile order hh*8+i <-> global head 2i+hh (wout is
                # permuted host-side to match)
                nc.sync.dma_start(
                    lhs[:, hh * NCORES:(hh + 1) * NCORES, :],
                    obs[(b, hh)][:].rearrange("i p s -> p i s"))

            wos = {}

            def op_group(b, n, m, lhs, pool=None):
                def emit():
                    wo = wos[n]
                    pl = pool if pool is not None else psF
                    ps = pl.tile([128, 512], f32,
                                 tag="fbank" if pl is psF else "bank")
                    for k in range(KSUB):
                        nc.tensor.matmul(
                            ps[:],
                            lhs[:, k, m * 128:(m + 1) * 128],
                            wo[:, k],
                            start=(k == 0), stop=(k == KSUB - 1))
                    ys = yp.tile([128, 512], f32, tag="ys", name="ys")
                    nc.vector.tensor_copy(ys[:], ps[:])
                    nc.scalar.dma_start(
                        y.ap()[b, m * 128:(m + 1) * 128,
                               n * 512:(n + 1) * 512],
                        ys[:])
                return emit

            # ---- schedule ----
            qkT0, Vn0 = qkv_rope(0, xc0)
            attention(0, 0, qkT0, Vn0)
            a2a(0, 0)
            lhs0 = lp.tile([128, KSUB, SCW], bf16, tag="lhs", name="lhs0")
            load_lhs_part(0, 0, lhs0)
            # pre-issue batch-1 chunk-0 x load (runs during attention(0,1))
            xc10 = xp.tile([128, KSUB, SC], bf16, tag="xc", name="xc")
            nc.gpsimd.dma_start(xc10[:], xT.ap()[:, :, S:S + SC])
            attention(0, 1, qkT0, Vn0)
            a2a(0, 1)
            load_lhs_part(0, 1, lhs0)
            qkT1, Vn1 = qkv_rope(1, xc10)
            # prefetch all of w_out on the sync queue: issued once a2a(0,1)
            # completes (during attention(1,0)), ready before the fillers.
            for n in range(4):
                wo = wop.tile([128, KSUB, 512], bf16, tag="wo",
                              name=f"wo{n}")
                nc.sync.dma_start(wo[:],
                                  wout.ap()[:, :, n * 512:(n + 1) * 512])
                wos[n] = wo
            attention(1, 0, qkT1, Vn1)
            a2a(1, 0)
            lhs1 = lp.tile([128, KSUB, SCW], bf16, tag="lhs", name="lhs1")
            load_lhs_part(1, 0, lhs1)
            fillers = [op_group(0, n, m, lhs0)
                       for n in range(4) for m in range(SCW // 128)]
            attention(1, 1, qkT1, Vn1, fillers)
            a2a(1, 1)
            # outproj(1) split: the k=0..7 (local head 0) halves depend only
            # on a2a(1,0), so they run during a2a(1,1)'s barrier wait; the
            # halves are staged to bf16 SBUF partials and completed with
            # k=8..15 once lhs1's second half lands.
            pools = [psF, psA]
            nms = [(n, m) for n in range(4) for m in range(SCW // 128)]
            parts = {}
            for gi, (n, m) in enumerate(nms):
                ps = pools[gi % 2].tile(
                    [128, 512], f32,
                    tag="fbank" if gi % 2 == 0 else "bank")
                for k in range(KSUB // 2):
                    nc.tensor.matmul(
                        ps[:],
                        lhs1[:, k, m * 128:(m + 1) * 128],
                        wos[n][:, k],
                        start=(k == 0), stop=(k == KSUB // 2 - 1))
                part = pp.tile([128, 512], bf16, tag="part", name="part")
                nc.vector.tensor_copy(part[:], ps[:])
                parts[(n, m)] = part
            load_lhs_part(1, 1, lhs1)
            for gi, (n, m) in enumerate(nms):
                ps = pools[gi % 2].tile(
                    [128, 512], f32,
                    tag="fbank" if gi % 2 == 0 else "bank")
                for k in range(KSUB // 2, KSUB):
                    nc.tensor.matmul(
                        ps[:],
                        lhs1[:, k, m * 128:(m + 1) * 128],
                        wos[n][:, k],
                        start=(k == KSUB // 2), stop=(k == KSUB - 1))
                ys = yp.tile([128, 512], f32, tag="ys", name="ys")
                nc.vector.tensor_add(ys[:], ps[:], parts[(n, m)][:])
                nc.scalar.dma_start(
                    y.ap()[1, m * 128:(m + 1) * 128,
                           n * 512:(n + 1) * 512],
                    ys[:])

    nc.finalize()
    return nc


def _host_inputs(x, w_qkv, w_out):
    xTr = np.ascontiguousarray(
        x.reshape(B * S, D).T.reshape(KSUB, 128, B * S).transpose(1, 0, 2)
    ).astype(BF16)
    horder = [2 * i + hh for hh in range(HPC) for i in range(NCORES)]
    woutr = np.ascontiguousarray(
        w_out.reshape(H, HD, D)[horder].transpose(1, 0, 2)).astype(BF16)

    half = HD // 2
    inv = (1.0 / (ROPE_BASE ** (np.arange(half, dtype=np.float32) / half))
           ).astype(np.float32)
    ang = (np.arange(S, dtype=np.float32)[:, None] * inv[None, :])  # [S, 64]
    c = np.cos(ang).astype(np.float32).T      # [64, S]
    s = np.sin(ang).astype(np.float32).T
    cosg = np.ascontiguousarray(np.concatenate([c, c], axis=0)).astype(BF16)
    # pre-swapped: rows 0:64 = +sin (consumed against t[0:64] -> rt[64:128]),
    # rows 64:128 = -sin (consumed against t[64:128] -> rt[0:64])
    sing = np.ascontiguousarray(np.concatenate([s, -s], axis=0)).astype(BF16)

    # mneg[p, j] = 0 where j >= p else -1e9 (upper-tri of the diagonal
    # 128-block, padded to 512 query columns).
    u = np.arange(512)[None, :]
    p = np.arange(128)[:, None]
    mneg = np.where(u >= p, 0.0, -1e9).astype(BF16)

    maps = []
    for i in range(NCORES):
        h0, h1 = 2 * i, 2 * i + 1
        blocks = []
        for base in (0, D, 2 * D):
            blocks.append(w_qkv[:, base + 128 * h0:base + 128 * (h0 + 1)])
            blocks.append(w_qkv[:, base + 128 * h1:base + 128 * (h1 + 1)])
        shard = np.concatenate(blocks, axis=1)  # [D, 768]
        shard = np.ascontiguousarray(
            shard.reshape(KSUB, 128, 3 * HPC * HD).transpose(1, 0, 2)
        ).astype(BF16)
        maps.append({"xT": xTr, "wqkv": shard, "wout": woutr,
                     "cosg": cosg, "sing": sing, "mneg": mneg})
    return maps


def kernel(x, w_qkv, w_out):
    from concourse.bass_utils import run_bass_kernel_spmd

    x = np.asarray(x, dtype=np.float32)
    w_qkv = np.asarray(w_qkv, dtype=np.float32)
    w_out = np.asarray(w_out, dtype=np.float32)

    if "nc" not in _CACHE:
        _CACHE["nc"] = _build()
    nc = _CACHE["nc"]

    trace = bool(int(os.environ.get("KERNEL_TRACE", "0")))
    if trace:
        trace = _install_trace_shim()

    in_maps = _host_inputs(x, w_qkv, w_out)
    kw = {}
    if trace and bool(int(os.environ.get("KERNEL_TRACE_ALL", "0"))):
        kw = {"trace_cores": list(range(NCORES)), "stitch_traces": True}
    res = run_bass_kernel_spmd(nc, in_maps, core_ids=list(range(NCORES)),
                               trace=trace, **kw)
    _CACHE["last_result"] = res
    # y per core i: [B, 256, D] = output rows [b*2048 + i*256, +256)
    full = np.empty((B * S, D), dtype=np.float32)
    for i in range(NCORES):
        yi = res.results[i]["y"]
        for b in range(B):
            full[b * S + i * SCW: b * S + (i + 1) * SCW] = yi[b]
    return full.reshape(B, S, D)


# revision 31
# speedup vs baseline: 1.0311x; 1.0311x over previous
"""Trainium2 Bass kernel for causal multi-head attention with RoPE.

Problem: x[2,2048,2048] -> qkv proj -> RoPE(q,k) -> causal softmax attention
(16 heads, hd=128) -> out proj.  Sharding: tensor-parallel over heads
(2 heads/core x 8 cores); the output projection contraction is restored
with one AllToAll per (batch, head) (head-shards -> sequence-shards), so
each core computes a disjoint [2, 256, 2048] slice of the final output.

v2: all matmul operands are bf16 (PSUM accumulation stays fp32), which
makes LDWEIGHTS (107ns) hide completely under N=512 matmuls and halves
the AllToAll payload.  The causal mask is applied post-exp with a DVE
affine_select on the diagonal 128-block (no PE mask matmuls).  Softmax
denominators: PE ones-matmul partition-reduce -> DVE reciprocal ->
gpsimd partition_broadcast (no PE broadcast matmul).  RoPE is applied
per 512-chunk right after each projection copy so attention starts
almost immediately after the last chunk.  Batch-0's output projection
runs as filler groups inside batch-1's second attention head; all of
w_out is prefetched during attention so the tail is only the last
AllToAll + batch-1's projection.
"""

import os
import sys

if "/opt/trn_rl_repo" not in sys.path:
    sys.path.insert(0, "/opt/trn_rl_repo")

import numpy as np
import ml_dtypes

BF16 = ml_dtypes.bfloat16

B, S, D = 2, 2048, 2048
H, HD = 16, 128
NCORES = 8
HPC = H // NCORES          # heads per core (2)
ROPE_BASE = 10000.0
SCALE = 1.0 / float(np.sqrt(HD))
SC = 512                   # QKV matmul free-dim chunk (s positions)
KSUB = D // 128            # 16 contraction subtiles
SCW = S // NCORES          # 256: per-core output cols per batch
NQC = S // SC              # 4 qkv s-chunks per batch
NKT = S // 128             # 16 key tiles
VOFF = 2 * HPC * HD        # v block column offset in w shard (512)
FILL_KTS = (4, 5, 7, 8, 9, 11, 12, 13, 15)   # filler slots (avoid finalize kts)

_CACHE = {}


def _install_trace_shim():
    """Optionally register the axon NTFF profile hook (for test.py tracing)."""
    try:
        import types

        if "antenv.axon_hooks" in sys.modules:
            return True
        import antenv
        from trn_agent_boot.trn_boot import _ntff_profile_via_ctypes

        hook = _ntff_profile_via_ctypes("/opt/axon/libaxon_pjrt.so")
        mod = types.ModuleType("antenv.axon_hooks")
        _state = {"hook": hook}
        mod.get_axon_ntff_profile_hook = lambda: _state["hook"]
        mod.set_axon_ntff_profile_hook = lambda h: _state.__setitem__("hook", h)
        sys.modules["antenv.axon_hooks"] = mod
        antenv.axon_hooks = mod
        return True
    except Exception:
        return False


def _build():
    import concourse.bass as bass  # noqa: F401
    import concourse.mybir as mybir
    import concourse.tile as tile
    from concourse import bacc

    f32 = mybir.dt.float32
    f32r = mybir.dt.float32r
    bf16 = mybir.dt.bfloat16
    EXP = mybir.ActivationFunctionType.Exp

    nc = bacc.Bacc("TRN2", target_bir_lowering=False, debug=False,
                   num_devices=NCORES)

    # x, pre-transposed and chunk-major: [p, chunk=b*NQC+sc, k-subtile, s]
    # so one 512-position chunk load is 16KB-contiguous per partition.
    xT = nc.dram_tensor("xT", [128, B * NQC, KSUB, SC], bf16,
                        kind="ExternalInput")
    wqkv = nc.dram_tensor("wqkv", [128, KSUB, 3 * HPC * HD], bf16,
                          kind="ExternalInput")
    wout = nc.dram_tensor("wout", [128, KSUB, D], bf16, kind="ExternalInput")
    cosg = nc.dram_tensor("cosg", [128, S], bf16, kind="ExternalInput")
    sing = nc.dram_tensor("sing", [128, S], bf16, kind="ExternalInput")
    mneg = nc.dram_tensor("mneg", [128, 512], bf16, kind="ExternalInput")
    y = nc.dram_tensor("y", [B, SCW, D], f32, kind="ExternalOutput")

    with tile.TileContext(nc) as tc:
        with tc.tile_pool(name="const", bufs=1) as cp, \
             tc.tile_pool(name="dram", bufs=1, space="DRAM") as dp, \
             tc.tile_pool(name="psO", bufs=1, space="PSUM") as psO, \
             tc.tile_pool(name="psA", bufs=2, space="PSUM") as psA, \
             tc.tile_pool(name="psD", bufs=1, space="PSUM") as psD, \
             tc.tile_pool(name="psF", bufs=1, space="PSUM") as psF, \
             tc.tile_pool(name="w", bufs=1) as wp, \
             tc.tile_pool(name="xc", bufs=2) as xp, \
             tc.tile_pool(name="qkv", bufs=1) as qp, \
             tc.tile_pool(name="attn", bufs=1) as ap_, \
             tc.tile_pool(name="stp", bufs=1) as stp, \
             tc.tile_pool(name="rot", bufs=1) as rp, \
             tc.tile_pool(name="exp", bufs=6) as ep, \
             tc.tile_pool(name="row", bufs=2) as sp_, \
             tc.tile_pool(name="lhs", bufs=2) as lp, \
             tc.tile_pool(name="wo", bufs=4) as wop, \
             tc.tile_pool(name="part", bufs=8) as pp, \
             tc.tile_pool(name="ys", bufs=2) as yp:

            from concourse.masks import make_identity

            cos_sb = cp.tile([128, S], bf16, name="cos_sb")
            sin_sb = cp.tile([128, S], bf16, name="sin_sb")
            mneg_sb = cp.tile([128, 512], bf16, name="mneg_sb")
            identF = cp.tile([128, 128], f32, name="identF")
            identB = cp.tile([128, 128], bf16, name="identB")
            onesc = cp.tile([128, 1], f32, name="onesc")
            onescR = cp.tile([128, 1], f32r, name="onescR")
            onesr = cp.tile([1, 128], bf16, name="onesr")
            nc.vector.memset(onesc[:], 1.0)
            nc.vector.tensor_copy(onescR[:], onesc[:])
            nc.vector.memset(onesr[:], 1.0)
            make_identity(nc, identF[:])
            nc.vector.tensor_copy(identB[:], identF[:])

            # batch-0 chunk-0 x and the whole qkv weight shard: both issued
            # from gpsimd (software DGE — descriptor generation is ~20x
            # faster than the sync queue's HWDGE for 2048-descriptor loads).
            xc0 = xp.tile([128, KSUB, SC], bf16, tag="xc", name="xc")
            nc.gpsimd.dma_start(xc0[:], xT.ap()[:, 0])
            wAll = wp.tile([128, KSUB, 3 * HPC * HD], bf16, name="wAll")
            nc.gpsimd.dma_start(wAll[:], wqkv.ap())
            nc.scalar.dma_start(cos_sb[:], cosg.ap())
            nc.scalar.dma_start(sin_sb[:], sing.ap())
            nc.scalar.dma_start(mneg_sb[:], mneg.ap())

            ibs = {(b, h): dp.tile([NCORES, 128, SCW], bf16, name=f"ib{b}{h}")
                   for b in range(B) for h in range(HPC)}
            obs = {(b, h): dp.tile([NCORES, 128, SCW], bf16, name=f"ob{b}{h}")
                   for b in range(B) for h in range(HPC)}

            def qkv_rope(b, pre_xc):
                qkT = qp.tile([128, 2 * HPC, S], bf16, tag="qkT")
                Vn = qp.tile([128, NKT, HPC * HD], bf16, tag="Vn")
                xcs = [pre_xc]
                for sc in range(NQC):
                    xc = xcs[sc]
                    if sc + 1 < NQC:
                        # prefetch next chunk (gpsimd queue, one ahead)
                        nxt = xp.tile([128, KSUB, SC], bf16, tag="xc",
                                      name="xc")
                        nc.gpsimd.dma_start(nxt[:],
                                            xT.ap()[:, b * NQC + sc + 1])
                        xcs.append(nxt)
                    sl = slice(sc * SC, (sc + 1) * SC)
                    for m in (0, 2, 1, 3):   # q0, k0, q1, k1
                        ps = psA.tile([128, 512], f32, tag="bank")
                        for k in range(KSUB):
                            nc.tensor.matmul(
                                ps[:, :SC],
                                wAll[:, k, m * 128:(m + 1) * 128],
                                xc[:, k],
                                start=(k == 0), stop=(k == KSUB - 1))
                        nc.vector.tensor_copy(qkT[:, m, sl], ps[:, :SC])
                        # RoPE, fused halves (sin grid stored pre-swapped):
                        # rt[0:64] = t[64:128]*(-sin); rt[64:128] = t[0:64]*sin
                        # t *= cos; t += rt
                        rt = rp.tile([128, SC], bf16, tag="rot", name="rt")
                        nc.vector.tensor_mul(rt[0:64, :],
                                             qkT[64:128, m, sl],
                                             sin_sb[64:128, sl])
                        nc.vector.tensor_mul(rt[64:128, :],
                                             qkT[0:64, m, sl],
                                             sin_sb[0:64, sl])
                        nc.vector.tensor_mul(qkT[:, m, sl], qkT[:, m, sl],
                                             cos_sb[:, sl])
                        nc.vector.tensor_add(qkT[:, m, sl], qkT[:, m, sl],
                                             rt[:])
                    for st2 in range(SC // 128):
                        ps = psA.tile([128, 512], f32, tag="bank")
                        for k in range(KSUB):
                            nc.tensor.matmul(
                                ps[:, :HPC * HD],
                                xc[:, k, st2 * 128:(st2 + 1) * 128],
                                wAll[:, k, VOFF:VOFF + HPC * HD],
                                start=(k == 0), stop=(k == KSUB - 1))
                        nc.vector.tensor_copy(Vn[:, sc * (SC // 128) + st2],
                                              ps[:, :HPC * HD])
                return qkT, Vn

            def attention(b, h, qkT, Vn, fillers=()):
                fillers = list(fillers)
                outT = psO.tile([128, S], f32, tag="outT")
                acc = ap_.tile([128, S], f32r, tag="acc")
                st = stp.tile([128, S], bf16, tag="st")

                def emit_av(kt, off, ets):
                    q0 = 512 * (kt // 4)
                    for c in range(len(ets)):
                        qs = q0 + c * 512
                        o = off if c == 0 else 0
                        nc.tensor.matmul(
                            outT[:, qs + o:qs + 512],
                            Vn[:, kt, h * 128:(h + 1) * 128],
                            ets[c][:, o:512],
                            start=(kt == 0),
                            stop=(kt == 4 * (qs // 512) + 3))

                def finalize_chunk(j):
                    # denom: partition-reduce ones-matmul, fp32 reciprocal,
                    # K=1 broadcast matmul, normalize, ship to DRAM.
                    sl = slice(j * 512, (j + 1) * 512)
                    rps = psD.tile([128, 512], f32, tag="dps", name="rps")
                    nc.tensor.matmul(rps[0:1, :], onescR[:], acc[:, sl],
                                     start=True, stop=True)
                    srow = sp_.tile([1, 512], f32, tag="srow")
                    nc.vector.reciprocal_approx_fast(srow[:], rps[0:1, :])
                    srb = sp_.tile([1, 512], bf16, tag="srb")
                    nc.vector.tensor_copy(srb[:], srow[:])
                    bp = psD.tile([128, 512], f32, tag="dps", name="bp")
                    nc.tensor.matmul(bp[:], onesr[:], srb[:],
                                     start=True, stop=True)
                    nc.vector.tensor_copy(st[:, sl], outT[:, sl])
                    nc.vector.tensor_mul(st[:, sl], st[:, sl], bp[:])
                    for jj in (2 * j, 2 * j + 1):
                        nc.gpsimd.dma_start(ibs[(b, h)][jj],
                                            st[:, jj * SCW:(jj + 1) * SCW])

                def emit_score(kt, c, off):
                    q0 = 512 * (kt // 4)
                    qs = q0 + c * 512
                    o = off if c == 0 else 0
                    sp = psA.tile([128, 512], f32, tag="bank")
                    if c == 0:
                        # -1e9 upper-tri mask for the diagonal 128 block
                        nc.tensor.matmul(sp[:, o:512], identB[:],
                                         mneg_sb[:, 0:512 - o],
                                         start=True, stop=False)
                    nc.tensor.matmul(
                        sp[:, o:512],
                        qkT[:, HPC + h, kt * 128:(kt + 1) * 128],
                        qkT[:, h, qs + o:qs + 512],
                        start=(c != 0), stop=True)
                    return sp

                prev = None
                for kt in range(NKT):
                    q0 = 512 * (kt // 4)
                    off = 128 * (kt % 4)
                    nch = (S - q0) // 512
                    sps = [emit_score(kt, c, off)
                           for c in range(min(nch, 2))]
                    if prev is not None:
                        emit_av(*prev)
                    sps += [emit_score(kt, c, off)
                            for c in range(2, nch)]
                    if kt >= 6 and (kt - 6) % 4 == 0:
                        finalize_chunk((kt - 6) // 4)
                    ets = []
                    for c in range(nch):
                        o = off if c == 0 else 0
                        et = ep.tile([128, 512], bf16, tag="expT")
                        ets.append(et)
                        nc.scalar.activation(et[:, o:512], sps[c][:, o:512],
                                             EXP, scale=SCALE)
                    for c in range(nch):
                        qs = q0 + c * 512
                        o = off if c == 0 else 0
                        if kt == 0:
                            nc.vector.tensor_copy(acc[:, qs:qs + 512],
                                                  ets[c][:])
                        else:
                            eng = nc.gpsimd if qs // 512 < 2 else nc.vector
                            eng.tensor_add(acc[:, qs + o:qs + 512],
                                           acc[:, qs + o:qs + 512],
                                           ets[c][:, o:512])
                    if fillers and kt in FILL_KTS:
                        fillers.pop(0)()
                    prev = (kt, off, ets)
                emit_av(*prev)
                finalize_chunk(3)
                while fillers:
                    fillers.pop(0)()

            def a2a(b, h):
                nc.gpsimd.collective_compute(
                    "AllToAll", mybir.AluOpType.bypass,
                    replica_groups=[list(range(NCORES))],
                    ins=[ibs[(b, h)].opt()], outs=[obs[(b, h)].opt()])

            def load_lhs_part(b, hh, lhs):
                # k-subtile order hh*8+i <-> global head 2i+hh (wout is
                # permuted host-side to match)
                nc.sync.dma_start(
                    lhs[:, hh * NCORES:(hh + 1) * NCORES, :],
                    obs[(b, hh)][:].rearrange("i p s -> p i s"))

            wos = {}

            def op_group(b, n, m, lhs, pool=None):
                def emit():
                    wo = wos[n]
                    pl = pool if pool is not None else psF
                    ps = pl.tile([128, 512], f32,
                                 tag="fbank" if pl is psF else "bank")
                    for k in range(KSUB):
                        nc.tensor.matmul(
                            ps[:],
                            lhs[:, k, m * 128:(m + 1) * 128],
                            wo[:, k],
                            start=(k == 0), stop=(k == KSUB - 1))
                    ys = yp.tile([128, 512], f32, tag="ys", name="ys")
                    nc.vector.tensor_copy(ys[:], ps[:])
                    nc.scalar.dma_start(
                        y.ap()[b, m * 128:(m + 1) * 128,
                               n * 512:(n + 1) * 512],
                        ys[:])
                return emit

            # ---- schedule ----
            qkT0, Vn0 = qkv_rope(0, xc0)
            attention(0, 0, qkT0, Vn0)
            a2a(0, 0)
            lhs0 = lp.tile([128, KSUB, SCW], bf16, tag="lhs", name="lhs0")
            load_lhs_part(0, 0, lhs0)
            # pre-issue batch-1 chunk-0 x load (runs during attention(0,1))
            xc10 = xp.tile([128, KSUB, SC], bf16, tag="xc", name="xc")
            nc.gpsimd.dma_start(xc10[:], xT.ap()[:, NQC])
            attention(0, 1, qkT0, Vn0)
            a2a(0, 1)
            load_lhs_part(0, 1, lhs0)
            qkT1, Vn1 = qkv_rope(1, xc10)
            # prefetch all of w_out on the sync queue: issued once a2a(0,1)
            # completes (during attention(1,0)), ready before the fillers.
            for n in range(4):
                wo = wop.tile([128, KSUB, 512], bf16, tag="wo",
                              name=f"wo{n}")
                nc.sync.dma_start(wo[:],
                                  wout.ap()[:, :, n * 512:(n + 1) * 512])
                wos[n] = wo
            attention(1, 0, qkT1, Vn1)
            a2a(1, 0)
            lhs1 = lp.tile([128, KSUB, SCW], bf16, tag="lhs", name="lhs1")
            load_lhs_part(1, 0, lhs1)
            fillers = [op_group(0, n, m, lhs0)
                       for n in range(4) for m in range(SCW // 128)]
            attention(1, 1, qkT1, Vn1, fillers)
            a2a(1, 1)
            # outproj(1) split: the k=0..7 (local head 0) halves depend only
            # on a2a(1,0), so they run during a2a(1,1)'s barrier wait; the
            # halves are staged to bf16 SBUF partials and completed with
            # k=8..15 once lhs1's second half lands.
            pools = [psF, psA]
            nms = [(n, m) for n in range(4) for m in range(SCW // 128)]
            parts = {}
            for gi, (n, m) in enumerate(nms):
                ps = pools[gi % 2].tile(
                    [128, 512], f32,
                    tag="fbank" if gi % 2 == 0 else "bank")
                for k in range(KSUB // 2):
                    nc.tensor.matmul(
                        ps[:],
                        lhs1[:, k, m * 128:(m + 1) * 128],
                        wos[n][:, k],
                        start=(k == 0), stop=(k == KSUB // 2 - 1))
                part = pp.tile([128, 512], bf16, tag="part", name="part")
                nc.vector.tensor_copy(part[:], ps[:])
                parts[(n, m)] = part
            load_lhs_part(1, 1, lhs1)
            for gi, (n, m) in enumerate(nms):
                ps = pools[gi % 2].tile(
                    [128, 512], f32,
                    tag="fbank" if gi % 2 == 0 else "bank")
                for k in range(KSUB // 2, KSUB):
                    nc.tensor.matmul(
                        ps[:],
                        lhs1[:, k, m * 128:(m + 1) * 128],
                        wos[n][:, k],
                        start=(k == KSUB // 2), stop=(k == KSUB - 1))
                ys = yp.tile([128, 512], f32, tag="ys", name="ys")
                nc.vector.tensor_add(ys[:], ps[:], parts[(n, m)][:])
                nc.scalar.dma_start(
                    y.ap()[1, m * 128:(m + 1) * 128,
                           n * 512:(n + 1) * 512],
                    ys[:])

    nc.finalize()
    return nc


def _host_inputs(x, w_qkv, w_out):
    # [p, chunk, k-subtile, s]: chunk-contiguous per partition (16KB rows)
    xTr = np.ascontiguousarray(
        x.reshape(B * S, D).T.reshape(KSUB, 128, B * NQC, SC)
        .transpose(1, 2, 0, 3)).astype(BF16)
    horder = [2 * i + hh for hh in range(HPC) for i in range(NCORES)]
    woutr = np.ascontiguousarray(
        w_out.reshape(H, HD, D)[horder].transpose(1, 0, 2)).astype(BF16)

    half = HD // 2
    inv = (1.0 / (ROPE_BASE ** (np.arange(half, dtype=np.float32) / half))
           ).astype(np.float32)
    ang = (np.arange(S, dtype=np.float32)[:, None] * inv[None, :])  # [S, 64]
    c = np.cos(ang).astype(np.float32).T      # [64, S]
    s = np.sin(ang).astype(np.float32).T
    cosg = np.ascontiguousarray(np.concatenate([c, c], axis=0)).astype(BF16)
    # pre-swapped: rows 0:64 = +sin (consumed against t[0:64] -> rt[64:128]),
    # rows 64:128 = -sin (consumed against t[64:128] -> rt[0:64])
    sing = np.ascontiguousarray(np.concatenate([s, -s], axis=0)).astype(BF16)

    # mneg[p, j] = 0 where j >= p else -1e9 (upper-tri of the diagonal
    # 128-block, padded to 512 query columns).
    u = np.arange(512)[None, :]
    p = np.arange(128)[:, None]
    mneg = np.where(u >= p, 0.0, -1e9).astype(BF16)

    maps = []
    for i in range(NCORES):
        h0, h1 = 2 * i, 2 * i + 1
        blocks = []
        for base in (0, D, 2 * D):
            blocks.append(w_qkv[:, base + 128 * h0:base + 128 * (h0 + 1)])
            blocks.append(w_qkv[:, base + 128 * h1:base + 128 * (h1 + 1)])
        shard = np.concatenate(blocks, axis=1)  # [D, 768]
        shard = np.ascontiguousarray(
            shard.reshape(KSUB, 128, 3 * HPC * HD).transpose(1, 0, 2)
        ).astype(BF16)
        maps.append({"xT": xTr, "wqkv": shard, "wout": woutr,
                     "cosg": cosg, "sing": sing, "mneg": mneg})
    return maps


def kernel(x, w_qkv, w_out):
    from concourse.bass_utils import run_bass_kernel_spmd

    x = np.asarray(x, dtype=np.float32)
    w_qkv = np.asarray(w_qkv, dtype=np.float32)
    w_out = np.asarray(w_out, dtype=np.float32)

    if "nc" not in _CACHE:
        _CACHE["nc"] = _build()
    nc = _CACHE["nc"]

    trace = bool(int(os.environ.get("KERNEL_TRACE", "0")))
    if trace:
        trace = _install_trace_shim()

    in_maps = _host_inputs(x, w_qkv, w_out)
    kw = {}
    if trace and bool(int(os.environ.get("KERNEL_TRACE_ALL", "0"))):
        kw = {"trace_cores": list(range(NCORES)), "stitch_traces": True}
    res = run_bass_kernel_spmd(nc, in_maps, core_ids=list(range(NCORES)),
                               trace=trace, **kw)
    _CACHE["last_result"] = res
    # y per core i: [B, 256, D] = output rows [b*2048 + i*256, +256)
    full = np.empty((B * S, D), dtype=np.float32)
    for i in range(NCORES):
        yi = res.results[i]["y"]
        for b in range(B):
            full[b * S + i * SCW: b * S + (i + 1) * SCW] = yi[b]
    return full.reshape(B, S, D)
